# revision 19
# baseline (speedup 1.0000x reference)
"""Trainium2 Bass kernel for nn_DepthCue (dynamic-filter / CARAFE-style module).

Sharding: data-parallel over batch B=8 across the 8 NeuronCores (one sample
per core).

Per core, row-block pipelined over 8 blocks of 8 image rows:
  - guide network (3x3 convs C->64->64->C) + DCK (1x1 convs + BN/ReLU) run on
    TensorE in bf16 (keeps the PE HAM-warm, 2x the f32r rate). conv2/conv3
    accumulate two taps per matmul via a shifted duplicate of the input in
    partitions 64-127; conv1 packs two row-blocks via column tiling.
  - dynamic-filter apply: partitions = (row-pair, group); per-tap elementwise
    multiply on VectorE in bf16 (2x mode), tap accumulation via
    identity-matmul into PSUM (fp32); the residual x is the PSUM init.
  - block pipeline overlaps the VectorE apply of block b with the guide
    convs of blocks b+1/b+2 on TensorE.
"""

import numpy as np
import ml_dtypes

import concourse.bass as bass
import concourse.bacc as bacc
import concourse.mybir as mybir
from concourse import bass_utils
from concourse.tile import TileContext

F32 = mybir.dt.float32
BF16 = mybir.dt.bfloat16
MULT = mybir.AluOpType.mult
RELU = mybir.ActivationFunctionType.Relu
COPY = mybir.ActivationFunctionType.Copy

N_CORES = 8
C, H, W = 512, 64, 64
HID = 64          # guide-net hidden channels
RED = 128         # DCK reduction channels
G = 32            # groups
GC = 16           # channels per group
K = 7             # dynamic kernel size
NTAP = K * K      # 49
NBLK = 8          # row blocks (8 rows each)
PW = W + 2        # padded width for conv intermediates (66)
PHW = (H + 2) * PW  # 4356
PIX = H * W       # 4096
XGW = W + 6       # apply x padded cols (70)
XGR = H + 6       # apply x padded rows (70)
XGS_GC = XGR * XGW          # 4900
XGS_G = GC * XGS_GC         # 78400
XES = GC * 8 * XGW          # xe per-partition elems (gc, 8 rows, 70) = 8960
XGS_GC_E = 8 * XGW          # xe per-gc stride (560)
TPAD = 64                   # taps padded to 64 in fbuf
FB_G = TPAD * PIX           # fbuf per-group stride (262144)


def ap_of(t, offset, dims):
    """Raw AP over tile/dram tensor t: dims = [[step, count], ...] (dim0 = partition for sbuf)."""
    base = t if isinstance(t, bass.AP) else t[:]
    return bass.AP(tensor=base.tensor, offset=offset, ap=[list(d) for d in dims])


def build_nc():
    nc = bacc.Bacc(trn_type="TRN2", target_bir_lowering=False, debug=False)

    T = {}
    for cc in range(4):
        T[f"xc{cc}"] = nc.dram_tensor(f"xc{cc}", [128, PHW], BF16, kind="ExternalInput").ap()
    T["xg"] = nc.dram_tensor("xg", [G, GC, XGR, XGW], BF16, kind="ExternalInput").ap()
    T["xgo"] = nc.dram_tensor("xgo", [G, GC, XGR, XGW], BF16, kind="ExternalInput").ap()
    T["w1t"] = nc.dram_tensor("w1t", [128, 9 * 4 * HID], BF16, kind="ExternalInput").ap()
    T["b1"] = nc.dram_tensor("b1", [HID, 1], F32, kind="ExternalInput").ap()
    T["w2pt"] = nc.dram_tensor("w2pt", [128, 3 * HID], BF16, kind="ExternalInput").ap()
    T["w2st"] = nc.dram_tensor("w2st", [HID, 3 * HID], BF16, kind="ExternalInput").ap()
    T["b2"] = nc.dram_tensor("b2", [HID, 1], F32, kind="ExternalInput").ap()
    T["w3pt"] = nc.dram_tensor("w3pt", [128, 3 * C], BF16, kind="ExternalInput").ap()
    T["w3st"] = nc.dram_tensor("w3st", [HID, 3 * C], BF16, kind="ExternalInput").ap()
    T["b3"] = nc.dram_tensor("b3", [128, 4], F32, kind="ExternalInput").ap()
    T["dw1t"] = nc.dram_tensor("dw1t", [128, 4 * RED], BF16, kind="ExternalInput").ap()
    T["bnsc"] = nc.dram_tensor("bnsc", [RED, 1], F32, kind="ExternalInput").ap()
    T["bnsh"] = nc.dram_tensor("bnsh", [RED, 1], F32, kind="ExternalInput").ap()
    T["dw2t"] = nc.dram_tensor("dw2t", [RED, 16 * 128], BF16, kind="ExternalInput").ap()
    T["idb"] = nc.dram_tensor("idb", [128, 128], BF16, kind="ExternalInput").ap()
    T["out"] = nc.dram_tensor("out", [C, H, W], F32, kind="ExternalOutput").ap()
    # filters scratch: [g, tap(64), h, w] bf16
    T["fbuf"] = nc.dram_tensor("fbuf", [G, TPAD, H, W], BF16, kind="Internal").ap()

    with TileContext(nc) as tc:
        build_body(nc, tc, T)
    nc.compile()
    return nc


def crhs(src, r0, dy, dx, npart):
    """conv rhs: padded rows r0+dy.., 8 output rows, cols dx.., over npart partitions."""
    return ap_of(src, (r0 + dy) * PW + dx, [[PHW, npart], [PW, 8], [1, W]])


def build_body(nc, tc, T):
    out, fbuf, xg, xgo = T["out"], T["fbuf"], T["xg"], T["xgo"]

    with tc.tile_pool(name="wp", bufs=1) as wp:
        # ---- persistent weights ----
        w1s = wp.tile([128, 9 * 4 * HID], BF16)     # [ci%128, (tap, cc, co)]
        nc.sync.dma_start(w1s[:], T["w1t"][:])
        w2ps = wp.tile([128, 3 * HID], BF16)        # [dy: taps (3dy, 3dy+1) stacked]
        nc.sync.dma_start(w2ps[:], T["w2pt"][:])
        w2ss = wp.tile([HID, 3 * HID], BF16)        # [dy: tap 3dy+2]
        nc.sync.dma_start(w2ss[:], T["w2st"][:])
        w3ps = wp.tile([128, 3 * C], BF16)
        nc.sync.dma_start(w3ps[:], T["w3pt"][:])
        w3ss = wp.tile([HID, 3 * C], BF16)
        nc.sync.dma_start(w3ss[:], T["w3st"][:])
        dw1s = wp.tile([128, 4 * RED], BF16)        # [ci%128, (cc, co)]
        nc.sync.dma_start(dw1s[:], T["dw1t"][:])
        dw2s = wp.tile([RED, 16 * 128], BF16)       # [red, (mch: 2g x 64tap)]
        nc.sync.dma_start(dw2s[:], T["dw2t"][:])
        b1s = wp.tile([HID, 1], F32)
        nc.sync.dma_start(b1s[:], T["b1"][:])
        b2s = wp.tile([HID, 1], F32)
        nc.sync.dma_start(b2s[:], T["b2"][:])
        b3s = wp.tile([128, 4], F32)
        nc.sync.dma_start(b3s[:], T["b3"][:])
        bnscs = wp.tile([RED, 1], F32)
        nc.sync.dma_start(bnscs[:], T["bnsc"][:])
        bnshs = wp.tile([RED, 1], F32)
        nc.sync.dma_start(bnshs[:], T["bnsh"][:])
        idbs = wp.tile([128, 128], BF16)
        nc.sync.dma_start(idbs[:], T["idb"][:])

        # conv input (pre-padded bf16 from host)
        xcp = []
        for cc in range(4):
            t = wp.tile([128, PHW], BF16, name=f"xc{cc}")
            nc.sync.dma_start(t[:], T[f"xc{cc}"][:])
            xcp.append(t)

        # h1/h2 with shifted duplicate in partitions 64-127
        h1d = wp.tile([128, PHW], BF16)
        nc.gpsimd.memset(h1d[:].bitcast(F32), 0.0)
        h2d = wp.tile([128, PHW], BF16)
        nc.gpsimd.memset(h2d[:].bitcast(F32), 0.0)

        with (
            tc.tile_pool(name="gd", bufs=2) as gd,
            tc.tile_pool(name="tfp", bufs=2) as tfp,
            tc.tile_pool(name="fsp", bufs=2) as fsp,
            tc.tile_pool(name="xep", bufs=2) as xep,
            tc.tile_pool(name="xop", bufs=2) as xop,
            tc.tile_pool(name="ftp", bufs=2) as ftp,
            tc.tile_pool(name="ptp", bufs=3) as ptp,
            tc.tile_pool(name="obp", bufs=1) as obp,
            tc.tile_pool(name="cps", bufs=3, space="PSUM") as cps,
            tc.tile_pool(name="aps", bufs=1, space="PSUM") as aps,
        ):
            def conv1_pieces(pb):
                """conv1 for blocks 2pb (psum rows 0-63) and 2pb+1 (rows 64-127,
                col-tiled), split into small pieces for emission interleaving."""
                cell = {}

                def mk(cc, tg):
                    def piece():
                        if "ps" not in cell:
                            cell["ps"] = cps.tile([128, 512], F32, tag="cv", name=f"c1ps{pb}")
                        ps = cell["ps"]
                        for tap in range(3 * tg, 3 * tg + 3):
                            dy, dx = tap // 3, tap % 3
                            nmm = cc * 9 + tap
                            lt = w1s[:, (tap * 4 + cc) * HID:(tap * 4 + cc + 1) * HID]
                            nc.tensor.matmul(
                                ps[0:64, :], lt, crhs(xcp[cc], 16 * pb, dy, dx, 128),
                                start=(nmm == 0), stop=(nmm == 35),
                                tile_position=(0, 0), skip_group_check=True,
                            )
                            nc.tensor.matmul(
                                ps[64:128, :], lt, crhs(xcp[cc], 16 * pb + 8, dy, dx, 128),
                                start=(nmm == 0), stop=(nmm == 35),
                                tile_position=(0, 64), skip_group_check=True,
                            )
                    return piece

                def act_piece():
                    ps = cell["ps"]
                    for half in range(2):
                        r0 = 16 * pb + 8 * half
                        src = ps[64 * half:64 * half + 64, :]
                        nc.scalar.activation(
                            ap_of(h1d, (r0 + 1) * PW + 1, [[PHW, 64], [PW, 8], [1, W]]),
                            src, RELU, bias=b1s[:],
                        )
                        nc.scalar.activation(
                            ap_of(h1d, 64 * PHW + (r0 + 1) * PW, [[PHW, 64], [PW, 8], [1, W]]),
                            src, RELU, bias=b1s[:],
                        )

                return [mk(cc, tg) for cc in range(4) for tg in range(3)] + [act_piece]

            def conv2_pieces(b):
                def piece():
                    r0 = 8 * b
                    ps = cps.tile([128, 512], F32, tag="cv")
                    for dy in range(3):
                        nc.tensor.matmul(
                            ps[0:64, :], w2ps[:, dy * HID:(dy + 1) * HID],
                            ap_of(h1d, (r0 + dy) * PW, [[PHW, 128], [PW, 8], [1, W]]),
                            start=(dy == 0), stop=False,
                        )
                    for dy in range(3):
                        nc.tensor.matmul(
                            ps[0:64, :], w2ss[:, dy * HID:(dy + 1) * HID],
                            ap_of(h1d, (r0 + dy) * PW + 2, [[PHW, 64], [PW, 8], [1, W]]),
                            start=False, stop=(dy == 2),
                        )
                    nc.scalar.activation(
                        ap_of(h2d, (r0 + 1) * PW + 1, [[PHW, 64], [PW, 8], [1, W]]),
                        ps[0:64, :], RELU, bias=b2s[:],
                    )
                    nc.scalar.activation(
                        ap_of(h2d, 64 * PHW + (r0 + 1) * PW, [[PHW, 64], [PW, 8], [1, W]]),
                        ps[0:64, :], RELU, bias=b2s[:],
                    )
                return [piece]

            def conv3_pieces(b, gts):
                def mk(mc):
                    def piece():
                        r0 = 8 * b
                        ps = cps.tile([128, 512], F32, tag="cv")
                        for dy in range(3):
                            nc.tensor.matmul(
                                ps[:], w3ps[:, dy * C + mc * 128:dy * C + (mc + 1) * 128],
                                ap_of(h2d, (r0 + dy) * PW, [[PHW, 128], [PW, 8], [1, W]]),
                                start=(dy == 0), stop=False,
                            )
                        for dy in range(3):
                            nc.tensor.matmul(
                                ps[:], w3ss[:, dy * C + mc * 128:dy * C + (mc + 1) * 128],
                                ap_of(h2d, (r0 + dy) * PW + 2, [[PHW, 64], [PW, 8], [1, W]]),
                                start=False, stop=(dy == 2),
                            )
                        gt = gd.tile([128, 512], BF16, tag=f"g{mc}")
                        nc.scalar.activation(gt[:], ps[:], RELU, bias=b3s[:, mc:mc + 1])
                        gts.append(gt)
                    return piece
                return [mk(mc) for mc in range(4)]

            def dck_pieces(b, gts):
                cell = {}

                def dck1():
                    ps = cps.tile([128, 512], F32, tag="cv")
                    for ccc in range(4):
                        nc.tensor.matmul(
                            ps[:], dw1s[:, ccc * RED:(ccc + 1) * RED], gts[ccc][:],
                            start=(ccc == 0), stop=(ccc == 3),
                        )
                    tft = tfp.tile([RED, 512], BF16, tag="tf")
                    nc.scalar.activation(tft[:], ps[:], RELU, bias=bnshs[:], scale=bnscs[:])
                    cell["tf"] = tft

                def mk(mq):
                    def piece():
                        tft = cell["tf"]
                        fst = fsp.tile([128, 4 * 512], BF16, tag="fs")
                        for q in range(4):
                            m = 4 * mq + q
                            ps2 = cps.tile([128, 512], F32, tag="cv")
                            nc.tensor.matmul(
                                ps2[:], dw2s[:, m * 128:(m + 1) * 128], tft[:],
                                start=True, stop=True,
                            )
                            nc.scalar.activation(fst[:, q * 512:(q + 1) * 512], ps2[:], COPY)
                        nc.sync.dma_start(
                            ap_of(fbuf, (8 * mq) * FB_G + b * 512,
                                  [[FB_G, 2], [PIX, TPAD], [2 * FB_G, 4], [1, 512]]),
                            fst[:],
                        )
                    return piece

                return [dck1] + [mk(mq) for mq in range(4)]

            def load_pieces(b, nxt):
                def xld():
                    xe = xep.tile([128, XES], BF16, tag="xe")
                    xo = xop.tile([128, XES], BF16, tag="xo")
                    for rp in range(4):
                        nc.sync.dma_start(
                            xe[rp * 32:(rp + 1) * 32, :],
                            ap_of(xg, (b * 8 + 2 * rp) * XGW,
                                  [[XGS_G, G], [XGS_GC, GC], [XGW, 8], [1, XGW]]),
                        )
                        nc.sync.dma_start(
                            xo[rp * 32:(rp + 1) * 32, :],
                            ap_of(xgo, (b * 8 + 2 * rp) * XGW,
                                  [[XGS_G, G], [XGS_GC, GC], [XGW, 8], [1, XGW]]),
                        )
                    nxt["xe"], nxt["xo"] = xe, xo

                def mk_ft(t0c, t1c):
                    def piece():
                        ntc = t1c - t0c
                        ftt = ftp.tile([128, ntc * 128], BF16, tag=f"ft{t0c}")
                        for rp in range(4):
                            nc.sync.dma_start(
                                ftt[rp * 32:(rp + 1) * 32, :],
                                ap_of(fbuf, t0c * PIX + (b * 8 + 2 * rp) * W,
                                      [[FB_G, G], [PIX, ntc], [1, 128]]),
                            )
                        nxt.setdefault("ft", []).append(ftt)
                    return piece

                return [xld, mk_ft(0, 32), mk_ft(32, NTAP)]

            gts_by_block = {}

            def c3(b):
                return conv3_pieces(b, gts_by_block.setdefault(b, []))

            def dckb(b):
                return dck_pieces(b, gts_by_block[b])

            def stage_pieces(b, nxt):
                """Emission pieces interleaved into apply(b)'s tap loop. The
                filter chain runs two blocks ahead of the apply (D at b+2) so
                the dck2 -> fbuf -> ft round trip has a full block of slack;
                loads for b+1 are emitted before the b+2 fbuf writes so the
                DRAM dependency tracker orders them after the b+1 writes only."""
                pieces = []
                if b == 0:
                    pieces += conv1_pieces(2)
                    pieces += conv2_pieces(2) + conv2_pieces(3)
                    pieces += c3(1) + c3(2) + dckb(1)
                if b == 1:
                    pieces += conv1_pieces(3)
                if b + 4 < NBLK:
                    pieces += conv2_pieces(b + 4)
                if b + 3 < NBLK:
                    pieces += c3(b + 3)
                if b + 1 < NBLK:
                    pieces += load_pieces(b + 1, nxt)
                if b + 2 < NBLK:
                    pieces += dckb(b + 2)
                return pieces

            def apply_block(b, loaded, pieces):
                xe, xo, fts = loaded["xe"], loaded["xo"], loaded["ft"]
                pso = aps.tile([128, 2048], F32, tag="pso")
                # residual init: pso = I @ x_central
                for j in range(4):
                    nc.tensor.matmul(
                        pso[:, j * 512:(j + 1) * 512], idbs[:],
                        ap_of(xe, 3 * XGW + 3 + j * 4 * XGS_GC_E,
                              [[XES, 128], [XGS_GC_E, 4], [XGW, 2], [1, W]]),
                        start=True, stop=False,
                    )
                npc = 0
                for tch, (t0c, t1c) in enumerate(((0, 32), (32, NTAP))):
                    ntc = t1c - t0c
                    ftt = fts[tch]
                    for t in range(t0c, t1c):
                        dy, dx = t // K, t % K
                        if dx % 2 == 0:
                            xsrc, xoff = xe, dy * XGW + dx
                        else:
                            xsrc, xoff = xo, dy * XGW + dx - 1
                        in0 = ap_of(xsrc, xoff,
                                    [[XES, 128], [XGS_GC_E, GC], [XGW, 2], [1, W]])
                        in1 = ap_of(ftt, (t - t0c) * 128,
                                    [[ntc * 128, 128], [0, GC], [W, 2], [1, W]])
                        pt = ptp.tile([128, 2048], BF16, tag="pt")
                        pout = ap_of(pt, 0, [[2048, 128], [128, GC], [W, 2], [1, W]])
                        nc.vector.tensor_tensor(pout, in0, in1, op=MULT)
                        for j in range(4):
                            nc.tensor.matmul(
                                pso[:, j * 512:(j + 1) * 512], idbs[:],
                                pt[:, j * 512:(j + 1) * 512],
                                start=False, stop=(t == NTAP - 1),
                            )
                        # pace next-block stage emission across the tap loop
                        want = (t + 1) * len(pieces) // NTAP
                        while npc < want:
                            pieces[npc]()
                            npc += 1
                while npc < len(pieces):
                    pieces[npc]()
                    npc += 1
                ob = obp.tile([128, 2048], F32, tag="ob")
                nc.scalar.activation(ob[:], pso[:], COPY)
                # output stores on the gpsimd queue so the sync (load) queue
                # never head-of-line blocks on apply completion
                for rp in range(4):
                    nc.gpsimd.dma_start(
                        ap_of(out, (b * 8 + 2 * rp) * W,
                              [[GC * PIX, G], [PIX, GC], [W, 2], [1, W]]),
                        ob[rp * 32:(rp + 1) * 32, :],
                    )

            # ---- pipeline: minimal fill for block 0, then interleave ----
            loaded = {}
            for p in (conv1_pieces(0) + conv2_pieces(0) + conv1_pieces(1)
                      + conv2_pieces(1) + c3(0) + dckb(0) + load_pieces(0, loaded)):
                p()
            for b in range(NBLK):
                nxt = {}
                pieces = stage_pieces(b, nxt)
                apply_block(b, loaded, pieces)
                loaded = nxt


def prep_weights(inputs):
    """Host-side weight transforms shared by all cores."""
    bf = ml_dtypes.bfloat16
    w1 = np.asarray(inputs["w1"], np.float32)   # [64, 512, 3, 3]
    w2 = np.asarray(inputs["w2"], np.float32)
    w3 = np.asarray(inputs["w3"], np.float32)   # [512, 64, 3, 3]
    dck_w1 = np.asarray(inputs["dck_w1"], np.float32)  # [128, 512, 1, 1]
    dck_w2 = np.asarray(inputs["dck_w2"], np.float32)  # [1568, 128, 1, 1]

    def tapify(w):  # [co, ci, 3, 3] -> [9, ci, co]
        return np.ascontiguousarray(w.transpose(2, 3, 1, 0).reshape(9, w.shape[1], w.shape[0]))

    w1sb = tapify(w1).reshape(9, 4, 128, HID).transpose(2, 0, 1, 3).reshape(128, 9 * 4 * HID)

    def pair_split(w9):  # [9, ci(64), co] -> pair [128, 3*co], single [64, 3*co]
        co = w9.shape[2]
        wp = np.zeros((128, 3, co), np.float32)
        ws = np.zeros((64, 3, co), np.float32)
        for dy in range(3):
            wp[0:64, dy] = w9[3 * dy]
            wp[64:128, dy] = w9[3 * dy + 1]
            ws[:, dy] = w9[3 * dy + 2]
        return wp.reshape(128, 3 * co), ws.reshape(64, 3 * co)

    w2pb, w2sb = pair_split(tapify(w2))
    w3pb, w3sb = pair_split(tapify(w3))
    dw1sb = dck_w1.reshape(RED, C).T.reshape(4, 128, RED).transpose(1, 0, 2).reshape(128, 4 * RED)

    bn_g = np.asarray(inputs["bn_gamma"], np.float32)
    bn_b = np.asarray(inputs["bn_beta"], np.float32)
    bn_m = np.asarray(inputs["bn_mean"], np.float32)
    bn_v = np.asarray(inputs["bn_var"], np.float32)
    inv_std = bn_g / np.sqrt(bn_v + 1e-5)
    shift = bn_b - bn_m * inv_std

    dw2 = dck_w2.reshape(G, NTAP, RED)          # [g, t, red]
    dw2p = np.zeros((G, 64, RED), np.float32)
    dw2p[:, :NTAP] = dw2
    # per m-chunk: [red, 2g x 64t]
    dw2t = np.ascontiguousarray(dw2p.reshape(16, 128, RED).transpose(2, 0, 1).reshape(RED, 16 * 128))

    return {
        "w1t": w1sb.astype(bf),
        "b1": np.asarray(inputs["b1"], np.float32).reshape(HID, 1),
        "w2pt": w2pb.astype(bf),
        "w2st": w2sb.astype(bf),
        "b2": np.asarray(inputs["b2"], np.float32).reshape(HID, 1),
        "w3pt": w3pb.astype(bf),
        "w3st": w3sb.astype(bf),
        "b3": np.ascontiguousarray(np.asarray(inputs["b3"], np.float32).reshape(4, 128).T),
        "dw1t": dw1sb.astype(bf),
        "bnsc": inv_std.reshape(RED, 1),
        "bnsh": shift.reshape(RED, 1),
        "dw2t": dw2t.astype(bf),
        "idb": np.eye(128).astype(bf),
    }


def prep_x(xi):
    """Per-core x transforms: padded conv input + padded apply image (bf16)."""
    bf = ml_dtypes.bfloat16
    xi = np.asarray(xi, np.float32)
    xc = np.zeros((4, 128, H + 2, PW), np.float32)
    xc[:, :, 1:H + 1, 1:W + 1] = xi.reshape(4, 128, H, W)
    xgf = np.zeros((G, GC, XGR, XGW), np.float32)
    xgf[:, :, 3:H + 3, 3:W + 3] = xi.reshape(G, GC, H, W)
    xgo = np.zeros_like(xgf)
    xgo[:, :, :, :XGW - 1] = xgf[:, :, :, 1:]
    m = {f"xc{cc}": np.ascontiguousarray(xc[cc].reshape(128, PHW)).astype(bf) for cc in range(4)}
    m["xg"] = xgf.astype(bf)
    m["xgo"] = xgo.astype(bf)
    return m


def make_in_maps(inputs):
    wmap = prep_weights(inputs)
    x = np.asarray(inputs["x"], np.float32)
    return [{**prep_x(x[i]), **wmap} for i in range(N_CORES)]


_NC_CACHE = {}


def get_nc():
    if "nc" not in _NC_CACHE:
        _NC_CACHE["nc"] = build_nc()
    return _NC_CACHE["nc"]


def kernel(**inputs):
    nc = get_nc()
    in_maps = make_in_maps(inputs)
    res = bass_utils.run_bass_kernel_spmd(nc, in_maps, core_ids=list(range(N_CORES)))
    return np.stack([res.results[i]["out"] for i in range(N_CORES)]).astype(np.float32)


# revision 23
# speedup vs baseline: 1.2573x; 1.2573x over previous
"""Trainium2 Bass kernel for nn_DepthCue (dynamic-filter / CARAFE-style module).

Sharding: data-parallel over batch B=8 across the 8 NeuronCores (one sample
per core).

Per core, row-block pipelined over 8 blocks of 8 image rows:
  - guide network (3x3 convs C->64->64->C) + DCK (1x1 convs + BN/ReLU) run on
    TensorE in bf16 (keeps the PE HAM-warm, 2x the f32r rate). conv2/conv3
    accumulate two taps per matmul via a shifted duplicate of the input in
    partitions 64-127; conv1 packs two row-blocks via column tiling.
  - dynamic-filter apply: partitions = (row-pair, group); per-tap elementwise
    multiply on VectorE in bf16 (2x mode), tap accumulation via
    identity-matmul into PSUM (fp32); the residual x is the PSUM init.
  - block pipeline overlaps the VectorE apply of block b with the guide
    convs of blocks b+1/b+2 on TensorE.
"""

import numpy as np
import ml_dtypes

import concourse.bass as bass
import concourse.bacc as bacc
import concourse.mybir as mybir
from concourse import bass_utils
from concourse.tile import TileContext

F32 = mybir.dt.float32
BF16 = mybir.dt.bfloat16
MULT = mybir.AluOpType.mult
RELU = mybir.ActivationFunctionType.Relu
COPY = mybir.ActivationFunctionType.Copy

N_CORES = 8
C, H, W = 512, 64, 64
HID = 64          # guide-net hidden channels
RED = 128         # DCK reduction channels
G = 32            # groups
GC = 16           # channels per group
K = 7             # dynamic kernel size
NTAP = K * K      # 49
NBLK = 8          # row blocks (8 rows each)
PW = W + 2        # padded width for conv intermediates (66)
PHW = (H + 2) * PW  # 4356
PIX = H * W       # 4096
XGW = W + 6       # apply x padded cols (70)
XGR = H + 6       # apply x padded rows (70)
XGS_GC = XGR * XGW          # 4900
XGS_G = GC * XGS_GC         # 78400
XES = GC * 8 * XGW          # xe per-partition elems (gc, 8 rows, 70) = 8960
XGS_GC_E = 8 * XGW          # xe per-gc stride (560)
TPAD = 64                   # taps padded to 64 in fbuf
FB_G = TPAD * PIX           # fbuf per-group stride (262144)


def ap_of(t, offset, dims):
    """Raw AP over tile/dram tensor t: dims = [[step, count], ...] (dim0 = partition for sbuf)."""
    base = t if isinstance(t, bass.AP) else t[:]
    return bass.AP(tensor=base.tensor, offset=offset, ap=[list(d) for d in dims])


def build_nc():
    nc = bacc.Bacc(trn_type="TRN2", target_bir_lowering=False, debug=False)

    T = {}
    for cc in range(4):
        T[f"xc{cc}"] = nc.dram_tensor(f"xc{cc}", [128, PHW], BF16, kind="ExternalInput").ap()
    T["xsb"] = nc.dram_tensor("xsb", [NBLK, 128, XES], BF16, kind="ExternalInput").ap()
    T["xsbo"] = nc.dram_tensor("xsbo", [NBLK, 128, XES], BF16, kind="ExternalInput").ap()
    T["w1t"] = nc.dram_tensor("w1t", [128, 9 * 4 * HID], BF16, kind="ExternalInput").ap()
    T["b1"] = nc.dram_tensor("b1", [HID, 1], F32, kind="ExternalInput").ap()
    T["w2pt"] = nc.dram_tensor("w2pt", [128, 3 * HID], BF16, kind="ExternalInput").ap()
    T["w2st"] = nc.dram_tensor("w2st", [HID, 3 * HID], BF16, kind="ExternalInput").ap()
    T["b2"] = nc.dram_tensor("b2", [HID, 1], F32, kind="ExternalInput").ap()
    T["w3pt"] = nc.dram_tensor("w3pt", [128, 3 * C], BF16, kind="ExternalInput").ap()
    T["w3st"] = nc.dram_tensor("w3st", [HID, 3 * C], BF16, kind="ExternalInput").ap()
    T["b3"] = nc.dram_tensor("b3", [128, 4], F32, kind="ExternalInput").ap()
    T["dw1t"] = nc.dram_tensor("dw1t", [128, 4 * RED], BF16, kind="ExternalInput").ap()
    T["bnsc"] = nc.dram_tensor("bnsc", [RED, 1], F32, kind="ExternalInput").ap()
    T["bnsh"] = nc.dram_tensor("bnsh", [RED, 1], F32, kind="ExternalInput").ap()
    T["dw2t"] = nc.dram_tensor("dw2t", [RED, 16 * 128], BF16, kind="ExternalInput").ap()
    T["idb"] = nc.dram_tensor("idb", [128, 128], BF16, kind="ExternalInput").ap()
    T["out"] = nc.dram_tensor("out", [C, H, W], F32, kind="ExternalOutput").ap()
    # filters scratch: [g, tap(64), h, w] bf16
    T["fbuf"] = nc.dram_tensor("fbuf", [G, TPAD, H, W], BF16, kind="Internal").ap()

    with TileContext(nc) as tc:
        build_body(nc, tc, T)
    nc.compile()
    return nc


def crhs(src, r0, dy, dx, npart):
    """conv rhs: padded rows r0+dy.., 8 output rows, cols dx.., over npart partitions."""
    return ap_of(src, (r0 + dy) * PW + dx, [[PHW, npart], [PW, 8], [1, W]])


def build_body(nc, tc, T):
    out, fbuf, xsb, xsbo = T["out"], T["fbuf"], T["xsb"], T["xsbo"]

    with tc.tile_pool(name="wp", bufs=1) as wp:
        # ---- persistent weights ----
        w1s = wp.tile([128, 9 * 4 * HID], BF16)     # [ci%128, (tap, cc, co)]
        nc.sync.dma_start(w1s[:], T["w1t"][:])
        w2ps = wp.tile([128, 3 * HID], BF16)        # [dy: taps (3dy, 3dy+1) stacked]
        nc.sync.dma_start(w2ps[:], T["w2pt"][:])
        w2ss = wp.tile([HID, 3 * HID], BF16)        # [dy: tap 3dy+2]
        nc.sync.dma_start(w2ss[:], T["w2st"][:])
        w3ps = wp.tile([128, 3 * C], BF16)
        nc.sync.dma_start(w3ps[:], T["w3pt"][:])
        w3ss = wp.tile([HID, 3 * C], BF16)
        nc.sync.dma_start(w3ss[:], T["w3st"][:])
        dw1s = wp.tile([128, 4 * RED], BF16)        # [ci%128, (cc, co)]
        nc.sync.dma_start(dw1s[:], T["dw1t"][:])
        dw2s = wp.tile([RED, 16 * 128], BF16)       # [red, (mch: 2g x 64tap)]
        nc.sync.dma_start(dw2s[:], T["dw2t"][:])
        b1s = wp.tile([HID, 1], F32)
        nc.sync.dma_start(b1s[:], T["b1"][:])
        b2s = wp.tile([HID, 1], F32)
        nc.sync.dma_start(b2s[:], T["b2"][:])
        b3s = wp.tile([128, 4], F32)
        nc.sync.dma_start(b3s[:], T["b3"][:])
        bnscs = wp.tile([RED, 1], F32)
        nc.sync.dma_start(bnscs[:], T["bnsc"][:])
        bnshs = wp.tile([RED, 1], F32)
        nc.sync.dma_start(bnshs[:], T["bnsh"][:])
        idbs = wp.tile([128, 128], BF16)
        nc.sync.dma_start(idbs[:], T["idb"][:])

        # conv input (pre-padded bf16 from host)
        xcp = []
        for cc in range(4):
            t = wp.tile([128, PHW], BF16, name=f"xc{cc}")
            nc.sync.dma_start(t[:], T[f"xc{cc}"][:])
            xcp.append(t)

        # h1/h2 with shifted duplicate in partitions 64-127
        h1d = wp.tile([128, PHW], BF16)
        nc.gpsimd.memset(h1d[:].bitcast(F32), 0.0)
        h2d = wp.tile([128, PHW], BF16)
        nc.gpsimd.memset(h2d[:].bitcast(F32), 0.0)

        with (
            tc.tile_pool(name="gd", bufs=2) as gd,
            tc.tile_pool(name="tfp", bufs=2) as tfp,
            tc.tile_pool(name="fsp", bufs=2) as fsp,
            tc.tile_pool(name="xep", bufs=2) as xep,
            tc.tile_pool(name="xop", bufs=2) as xop,
            tc.tile_pool(name="ftp", bufs=2) as ftp,
            tc.tile_pool(name="ptp", bufs=3) as ptp,
            tc.tile_pool(name="obp", bufs=1) as obp,
            tc.tile_pool(name="cps", bufs=3, space="PSUM") as cps,
            tc.tile_pool(name="aps", bufs=1, space="PSUM") as aps,
        ):
            def conv1_pieces(pb):
                """conv1 for blocks 2pb (psum rows 0-63) and 2pb+1 (rows 64-127,
                col-tiled), split into small pieces for emission interleaving."""
                cell = {}

                def mk(cc, tg):
                    def piece():
                        if "ps" not in cell:
                            cell["ps"] = cps.tile([128, 512], F32, tag="cv", name=f"c1ps{pb}")
                        ps = cell["ps"]
                        for tap in range(3 * tg, 3 * tg + 3):
                            dy, dx = tap // 3, tap % 3
                            nmm = cc * 9 + tap
                            lt = w1s[:, (tap * 4 + cc) * HID:(tap * 4 + cc + 1) * HID]
                            nc.tensor.matmul(
                                ps[0:64, :], lt, crhs(xcp[cc], 16 * pb, dy, dx, 128),
                                start=(nmm == 0), stop=(nmm == 35),
                                tile_position=(0, 0), skip_group_check=True,
                            )
                            nc.tensor.matmul(
                                ps[64:128, :], lt, crhs(xcp[cc], 16 * pb + 8, dy, dx, 128),
                                start=(nmm == 0), stop=(nmm == 35),
                                tile_position=(0, 64), skip_group_check=True,
                            )
                    return piece

                def act_piece():
                    ps = cell["ps"]
                    for half in range(2):
                        r0 = 16 * pb + 8 * half
                        src = ps[64 * half:64 * half + 64, :]
                        nc.scalar.activation(
                            ap_of(h1d, (r0 + 1) * PW + 1, [[PHW, 64], [PW, 8], [1, W]]),
                            src, RELU, bias=b1s[:],
                        )
                        nc.scalar.activation(
                            ap_of(h1d, 64 * PHW + (r0 + 1) * PW, [[PHW, 64], [PW, 8], [1, W]]),
                            src, RELU, bias=b1s[:],
                        )

                return [mk(cc, tg) for cc in range(4) for tg in range(3)] + [act_piece]

            def conv2_pieces(b):
                def piece():
                    r0 = 8 * b
                    ps = cps.tile([128, 512], F32, tag="cv")
                    for dy in range(3):
                        nc.tensor.matmul(
                            ps[0:64, :], w2ps[:, dy * HID:(dy + 1) * HID],
                            ap_of(h1d, (r0 + dy) * PW, [[PHW, 128], [PW, 8], [1, W]]),
                            start=(dy == 0), stop=False,
                        )
                    for dy in range(3):
                        nc.tensor.matmul(
                            ps[0:64, :], w2ss[:, dy * HID:(dy + 1) * HID],
                            ap_of(h1d, (r0 + dy) * PW + 2, [[PHW, 64], [PW, 8], [1, W]]),
                            start=False, stop=(dy == 2),
                        )
                    nc.scalar.activation(
                        ap_of(h2d, (r0 + 1) * PW + 1, [[PHW, 64], [PW, 8], [1, W]]),
                        ps[0:64, :], RELU, bias=b2s[:],
                    )
                    nc.scalar.activation(
                        ap_of(h2d, 64 * PHW + (r0 + 1) * PW, [[PHW, 64], [PW, 8], [1, W]]),
                        ps[0:64, :], RELU, bias=b2s[:],
                    )
                return [piece]

            def conv3_pieces(b, gts):
                def mk(mc):
                    def piece():
                        r0 = 8 * b
                        ps = cps.tile([128, 512], F32, tag="cv")
                        for dy in range(3):
                            nc.tensor.matmul(
                                ps[:], w3ps[:, dy * C + mc * 128:dy * C + (mc + 1) * 128],
                                ap_of(h2d, (r0 + dy) * PW, [[PHW, 128], [PW, 8], [1, W]]),
                                start=(dy == 0), stop=False,
                            )
                        for dy in range(3):
                            nc.tensor.matmul(
                                ps[:], w3ss[:, dy * C + mc * 128:dy * C + (mc + 1) * 128],
                                ap_of(h2d, (r0 + dy) * PW + 2, [[PHW, 64], [PW, 8], [1, W]]),
                                start=False, stop=(dy == 2),
                            )
                        gt = gd.tile([128, 512], BF16, tag=f"g{mc}")
                        nc.scalar.activation(gt[:], ps[:], RELU, bias=b3s[:, mc:mc + 1])
                        gts.append(gt)
                    return piece
                return [mk(mc) for mc in range(4)]

            def dck_pieces(b, gts):
                cell = {}

                def dck1():
                    ps = cps.tile([128, 512], F32, tag="cv")
                    for ccc in range(4):
                        nc.tensor.matmul(
                            ps[:], dw1s[:, ccc * RED:(ccc + 1) * RED], gts[ccc][:],
                            start=(ccc == 0), stop=(ccc == 3),
                        )
                    tft = tfp.tile([RED, 512], BF16, tag="tf")
                    nc.scalar.activation(tft[:], ps[:], RELU, bias=bnshs[:], scale=bnscs[:])
                    cell["tf"] = tft

                def mk(mq):
                    def piece():
                        tft = cell["tf"]
                        fst = fsp.tile([128, 4 * 512], BF16, tag="fs")
                        for q in range(4):
                            m = 4 * mq + q
                            ps2 = cps.tile([128, 512], F32, tag="cv")
                            nc.tensor.matmul(
                                ps2[:], dw2s[:, m * 128:(m + 1) * 128], tft[:],
                                start=True, stop=True,
                            )
                            nc.scalar.activation(fst[:, q * 512:(q + 1) * 512], ps2[:], COPY)
                        nc.sync.dma_start(
                            ap_of(fbuf, (8 * mq) * FB_G + b * 512,
                                  [[FB_G, 2], [PIX, TPAD], [2 * FB_G, 4], [1, 512]]),
                            fst[:],
                        )
                    return piece

                return [dck1] + [mk(mq) for mq in range(4)]

            def load_pieces(b, nxt):
                def xld():
                    # contiguous per-block slabs (host-prepared): 128 fat
                    # descriptors per DMA instead of 4096 thin ones
                    xe = xep.tile([128, XES], BF16, tag="xe")
                    xo = xop.tile([128, XES], BF16, tag="xo")
                    nc.sync.dma_start(
                        xe[:], ap_of(xsb, b * 128 * XES, [[XES, 128], [1, XES]]))
                    nc.sync.dma_start(
                        xo[:], ap_of(xsbo, b * 128 * XES, [[XES, 128], [1, XES]]))
                    nxt["xe"], nxt["xo"] = xe, xo

                def mk_ft(t0c, t1c):
                    def piece():
                        # filter loads go on the scalar-engine HWDGE queue:
                        # ~1.5k thin descriptors per DMA would clog the sync
                        # queue's descriptor generator
                        ntc = t1c - t0c
                        ftt = ftp.tile([128, ntc * 128], BF16, tag=f"ft{t0c}")
                        for rp in range(4):
                            nc.scalar.dma_start(
                                ftt[rp * 32:(rp + 1) * 32, :],
                                ap_of(fbuf, t0c * PIX + (b * 8 + 2 * rp) * W,
                                      [[FB_G, G], [PIX, ntc], [1, 128]]),
                            )
                        nxt.setdefault("ft", []).append(ftt)
                    return piece

                return [xld, mk_ft(0, 32), mk_ft(32, NTAP)]

            gts_by_block = {}

            def c3(b):
                return conv3_pieces(b, gts_by_block.setdefault(b, []))

            def dckb(b):
                return dck_pieces(b, gts_by_block[b])

            def stage_pieces(b, nxt):
                """Emission pieces interleaved into apply(b)'s tap loop. The
                filter chain runs two blocks ahead of the apply (D at b+2) so
                the dck2 -> fbuf -> ft round trip has a full block of slack;
                loads for b+1 are emitted before the b+2 fbuf writes so the
                DRAM dependency tracker orders them after the b+1 writes only."""
                pieces = []
                if b == 0:
                    pieces += conv1_pieces(2)
                    pieces += conv2_pieces(2) + conv2_pieces(3)
                    pieces += c3(1) + c3(2) + dckb(1)
                if b == 1:
                    pieces += conv1_pieces(3)
                if b + 4 < NBLK:
                    pieces += conv2_pieces(b + 4)
                if b + 3 < NBLK:
                    pieces += c3(b + 3)
                if b + 1 < NBLK:
                    pieces += load_pieces(b + 1, nxt)
                if b + 2 < NBLK:
                    pieces += dckb(b + 2)
                return pieces

            def apply_block(b, loaded, pieces):
                xe, xo, fts = loaded["xe"], loaded["xo"], loaded["ft"]
                pso = aps.tile([128, 2048], F32, tag="pso")
                # residual init: pso = I @ x_central
                for j in range(4):
                    nc.tensor.matmul(
                        pso[:, j * 512:(j + 1) * 512], idbs[:],
                        ap_of(xe, 3 * XGW + 3 + j * 4 * XGS_GC_E,
                              [[XES, 128], [XGS_GC_E, 4], [XGW, 2], [1, W]]),
                        start=True, stop=False,
                    )
                npc = 0
                for tch, (t0c, t1c) in enumerate(((0, 32), (32, NTAP))):
                    ntc = t1c - t0c
                    ftt = fts[tch]
                    for t in range(t0c, t1c):
                        dy, dx = t // K, t % K
                        if dx % 2 == 0:
                            xsrc, xoff = xe, dy * XGW + dx
                        else:
                            xsrc, xoff = xo, dy * XGW + dx - 1
                        in0 = ap_of(xsrc, xoff,
                                    [[XES, 128], [XGS_GC_E, GC], [XGW, 2], [1, W]])
                        in1 = ap_of(ftt, (t - t0c) * 128,
                                    [[ntc * 128, 128], [0, GC], [W, 2], [1, W]])
                        pt = ptp.tile([128, 2048], BF16, tag="pt")
                        pout = ap_of(pt, 0, [[2048, 128], [128, GC], [W, 2], [1, W]])
                        nc.vector.tensor_tensor(pout, in0, in1, op=MULT)
                        for j in range(4):
                            nc.tensor.matmul(
                                pso[:, j * 512:(j + 1) * 512], idbs[:],
                                pt[:, j * 512:(j + 1) * 512],
                                start=False, stop=(t == NTAP - 1),
                            )
                        # pace next-block stage emission across the tap loop
                        want = (t + 1) * len(pieces) // NTAP
                        while npc < want:
                            pieces[npc]()
                            npc += 1
                while npc < len(pieces):
                    pieces[npc]()
                    npc += 1
                ob = obp.tile([128, 2048], F32, tag="ob")
                nc.scalar.activation(ob[:], pso[:], COPY)
                # output stores on the gpsimd queue so the sync (load) queue
                # never head-of-line blocks on apply completion
                for rp in range(4):
                    nc.gpsimd.dma_start(
                        ap_of(out, (b * 8 + 2 * rp) * W,
                              [[GC * PIX, G], [PIX, GC], [W, 2], [1, W]]),
                        ob[rp * 32:(rp + 1) * 32, :],
                    )

            # ---- pipeline: minimal fill for block 0, then interleave ----
            loaded = {}
            for p in (conv1_pieces(0) + conv2_pieces(0) + conv1_pieces(1)
                      + conv2_pieces(1) + c3(0) + dckb(0) + load_pieces(0, loaded)):
                p()
            for b in range(NBLK):
                nxt = {}
                pieces = stage_pieces(b, nxt)
                apply_block(b, loaded, pieces)
                loaded = nxt


def prep_weights(inputs):
    """Host-side weight transforms shared by all cores."""
    bf = ml_dtypes.bfloat16
    w1 = np.asarray(inputs["w1"], np.float32)   # [64, 512, 3, 3]
    w2 = np.asarray(inputs["w2"], np.float32)
    w3 = np.asarray(inputs["w3"], np.float32)   # [512, 64, 3, 3]
    dck_w1 = np.asarray(inputs["dck_w1"], np.float32)  # [128, 512, 1, 1]
    dck_w2 = np.asarray(inputs["dck_w2"], np.float32)  # [1568, 128, 1, 1]

    def tapify(w):  # [co, ci, 3, 3] -> [9, ci, co]
        return np.ascontiguousarray(w.transpose(2, 3, 1, 0).reshape(9, w.shape[1], w.shape[0]))

    w1sb = tapify(w1).reshape(9, 4, 128, HID).transpose(2, 0, 1, 3).reshape(128, 9 * 4 * HID)

    def pair_split(w9):  # [9, ci(64), co] -> pair [128, 3*co], single [64, 3*co]
        co = w9.shape[2]
        wp = np.zeros((128, 3, co), np.float32)
        ws = np.zeros((64, 3, co), np.float32)
        for dy in range(3):
            wp[0:64, dy] = w9[3 * dy]
            wp[64:128, dy] = w9[3 * dy + 1]
            ws[:, dy] = w9[3 * dy + 2]
        return wp.reshape(128, 3 * co), ws.reshape(64, 3 * co)

    w2pb, w2sb = pair_split(tapify(w2))
    w3pb, w3sb = pair_split(tapify(w3))
    dw1sb = dck_w1.reshape(RED, C).T.reshape(4, 128, RED).transpose(1, 0, 2).reshape(128, 4 * RED)

    bn_g = np.asarray(inputs["bn_gamma"], np.float32)
    bn_b = np.asarray(inputs["bn_beta"], np.float32)
    bn_m = np.asarray(inputs["bn_mean"], np.float32)
    bn_v = np.asarray(inputs["bn_var"], np.float32)
    inv_std = bn_g / np.sqrt(bn_v + 1e-5)
    shift = bn_b - bn_m * inv_std

    dw2 = dck_w2.reshape(G, NTAP, RED)          # [g, t, red]
    dw2p = np.zeros((G, 64, RED), np.float32)
    dw2p[:, :NTAP] = dw2
    # per m-chunk: [red, 2g x 64t]
    dw2t = np.ascontiguousarray(dw2p.reshape(16, 128, RED).transpose(2, 0, 1).reshape(RED, 16 * 128))

    return {
        "w1t": w1sb.astype(bf),
        "b1": np.asarray(inputs["b1"], np.float32).reshape(HID, 1),
        "w2pt": w2pb.astype(bf),
        "w2st": w2sb.astype(bf),
        "b2": np.asarray(inputs["b2"], np.float32).reshape(HID, 1),
        "w3pt": w3pb.astype(bf),
        "w3st": w3sb.astype(bf),
        "b3": np.ascontiguousarray(np.asarray(inputs["b3"], np.float32).reshape(4, 128).T),
        "dw1t": dw1sb.astype(bf),
        "bnsc": inv_std.reshape(RED, 1),
        "bnsh": shift.reshape(RED, 1),
        "dw2t": dw2t.astype(bf),
        "idb": np.eye(128).astype(bf),
    }


def prep_x(xi):
    """Per-core x transforms: padded conv input + padded apply image (bf16)."""
    bf = ml_dtypes.bfloat16
    xi = np.asarray(xi, np.float32)
    xc = np.zeros((4, 128, H + 2, PW), np.float32)
    xc[:, :, 1:H + 1, 1:W + 1] = xi.reshape(4, 128, H, W)
    xgf = np.zeros((G, GC, XGR, XGW), np.float32)
    xgf[:, :, 3:H + 3, 3:W + 3] = xi.reshape(G, GC, H, W)
    xgo = np.zeros_like(xgf)
    xgo[:, :, :, :XGW - 1] = xgf[:, :, :, 1:]
    # per-block contiguous slabs: [b][p=(rp,g)][(gc, 8 rows, 70)]
    xsb = np.empty((NBLK, 4, G, GC, 8, XGW), np.float32)
    xsbo = np.empty_like(xsb)
    for b in range(NBLK):
        for rp in range(4):
            r0 = b * 8 + 2 * rp
            xsb[b, rp] = xgf[:, :, r0:r0 + 8, :]
            xsbo[b, rp] = xgo[:, :, r0:r0 + 8, :]
    m = {f"xc{cc}": np.ascontiguousarray(xc[cc].reshape(128, PHW)).astype(bf) for cc in range(4)}
    m["xsb"] = xsb.reshape(NBLK, 128, XES).astype(bf)
    m["xsbo"] = xsbo.reshape(NBLK, 128, XES).astype(bf)
    return m


def make_in_maps(inputs):
    wmap = prep_weights(inputs)
    x = np.asarray(inputs["x"], np.float32)
    return [{**prep_x(x[i]), **wmap} for i in range(N_CORES)]


_NC_CACHE = {}


def get_nc():
    if "nc" not in _NC_CACHE:
        _NC_CACHE["nc"] = build_nc()
    return _NC_CACHE["nc"]


def kernel(**inputs):
    nc = get_nc()
    in_maps = make_in_maps(inputs)
    res = bass_utils.run_bass_kernel_spmd(nc, in_maps, core_ids=list(range(N_CORES)))
    return np.stack([res.results[i]["out"] for i in range(N_CORES)]).astype(np.float32)


# revision 25
# speedup vs baseline: 1.2814x; 1.0191x over previous
"""Trainium2 Bass kernel for nn_DepthCue (dynamic-filter / CARAFE-style module).

Sharding: data-parallel over batch B=8 across the 8 NeuronCores (one sample
per core).

Per core, row-block pipelined over 8 blocks of 8 image rows:
  - guide network (3x3 convs C->64->64->C) + DCK (1x1 convs + BN/ReLU) run on
    TensorE in bf16 (keeps the PE HAM-warm, 2x the f32r rate). conv2/conv3
    accumulate two taps per matmul via a shifted duplicate of the input in
    partitions 64-127; conv1 packs two row-blocks via column tiling.
  - dynamic-filter apply: partitions = (row-pair, group); per-tap elementwise
    multiply on VectorE in bf16 (2x mode), tap accumulation via
    identity-matmul into PSUM (fp32); the residual x is the PSUM init.
  - block pipeline overlaps the VectorE apply of block b with the guide
    convs of blocks b+1/b+2 on TensorE.
"""

import numpy as np
import ml_dtypes

import concourse.bass as bass
import concourse.bacc as bacc
import concourse.mybir as mybir
from concourse import bass_utils
from concourse.tile import TileContext

F32 = mybir.dt.float32
BF16 = mybir.dt.bfloat16
MULT = mybir.AluOpType.mult
RELU = mybir.ActivationFunctionType.Relu
COPY = mybir.ActivationFunctionType.Copy

N_CORES = 8
C, H, W = 512, 64, 64
HID = 64          # guide-net hidden channels
RED = 128         # DCK reduction channels
G = 32            # groups
GC = 16           # channels per group
K = 7             # dynamic kernel size
NTAP = K * K      # 49
NBLK = 8          # row blocks (8 rows each)
PW = W + 2        # padded width for conv intermediates (66)
PHW = (H + 2) * PW  # 4356
PIX = H * W       # 4096
XGW = W + 6       # apply x padded cols (70)
XGR = H + 6       # apply x padded rows (70)
XGS_GC = XGR * XGW          # 4900
XGS_G = GC * XGS_GC         # 78400
XES = GC * 8 * XGW          # xe per-partition elems (gc, 8 rows, 70) = 8960
XGS_GC_E = 8 * XGW          # xe per-gc stride (560)
TPAD = 64                   # taps padded to 64 in fbuf
FB_G = TPAD * PIX           # fbuf per-group stride (262144)


def ap_of(t, offset, dims):
    """Raw AP over tile/dram tensor t: dims = [[step, count], ...] (dim0 = partition for sbuf)."""
    base = t if isinstance(t, bass.AP) else t[:]
    return bass.AP(tensor=base.tensor, offset=offset, ap=[list(d) for d in dims])


def build_nc():
    nc = bacc.Bacc(trn_type="TRN2", target_bir_lowering=False, debug=False)

    T = {}
    for cc in range(4):
        T[f"xc{cc}"] = nc.dram_tensor(f"xc{cc}", [128, PHW], BF16, kind="ExternalInput").ap()
    T["xsb"] = nc.dram_tensor("xsb", [NBLK, 128, XES], BF16, kind="ExternalInput").ap()
    T["xsbo"] = nc.dram_tensor("xsbo", [NBLK, 128, XES], BF16, kind="ExternalInput").ap()
    T["w1t"] = nc.dram_tensor("w1t", [128, 9 * 4 * HID], BF16, kind="ExternalInput").ap()
    T["b1"] = nc.dram_tensor("b1", [HID, 1], F32, kind="ExternalInput").ap()
    T["w2pt"] = nc.dram_tensor("w2pt", [128, 3 * HID], BF16, kind="ExternalInput").ap()
    T["w2st"] = nc.dram_tensor("w2st", [HID, 3 * HID], BF16, kind="ExternalInput").ap()
    T["b2"] = nc.dram_tensor("b2", [HID, 1], F32, kind="ExternalInput").ap()
    T["w3pt"] = nc.dram_tensor("w3pt", [128, 3 * C], BF16, kind="ExternalInput").ap()
    T["w3st"] = nc.dram_tensor("w3st", [HID, 3 * C], BF16, kind="ExternalInput").ap()
    T["b3"] = nc.dram_tensor("b3", [128, 4], F32, kind="ExternalInput").ap()
    T["dw1t"] = nc.dram_tensor("dw1t", [128, 4 * RED], BF16, kind="ExternalInput").ap()
    T["bnsc"] = nc.dram_tensor("bnsc", [RED, 1], F32, kind="ExternalInput").ap()
    T["bnsh"] = nc.dram_tensor("bnsh", [RED, 1], F32, kind="ExternalInput").ap()
    T["dw2t"] = nc.dram_tensor("dw2t", [RED, 16 * 128], BF16, kind="ExternalInput").ap()
    T["idb"] = nc.dram_tensor("idb", [128, 128], BF16, kind="ExternalInput").ap()
    T["out"] = nc.dram_tensor("out", [C, H, W], F32, kind="ExternalOutput").ap()
    # filters scratch: [g, tap(64), h, w] bf16
    T["fbuf"] = nc.dram_tensor("fbuf", [G, TPAD, H, W], BF16, kind="Internal").ap()

    with TileContext(nc) as tc:
        build_body(nc, tc, T)
    nc.compile()
    return nc


def crhs(src, r0, dy, dx, npart):
    """conv rhs: padded rows r0+dy.., 8 output rows, cols dx.., over npart partitions."""
    return ap_of(src, (r0 + dy) * PW + dx, [[PHW, npart], [PW, 8], [1, W]])


def build_body(nc, tc, T):
    out, fbuf, xsb, xsbo = T["out"], T["fbuf"], T["xsb"], T["xsbo"]

    with tc.tile_pool(name="wp", bufs=1) as wp:
        # ---- persistent weights ----
        w1s = wp.tile([128, 9 * 4 * HID], BF16)     # [ci%128, (tap, cc, co)]
        nc.sync.dma_start(w1s[:], T["w1t"][:])
        w2ps = wp.tile([128, 3 * HID], BF16)        # [dy: taps (3dy, 3dy+1) stacked]
        nc.sync.dma_start(w2ps[:], T["w2pt"][:])
        w2ss = wp.tile([HID, 3 * HID], BF16)        # [dy: tap 3dy+2]
        nc.sync.dma_start(w2ss[:], T["w2st"][:])
        w3ps = wp.tile([128, 3 * C], BF16)
        nc.sync.dma_start(w3ps[:], T["w3pt"][:])
        w3ss = wp.tile([HID, 3 * C], BF16)
        nc.sync.dma_start(w3ss[:], T["w3st"][:])
        dw1s = wp.tile([128, 4 * RED], BF16)        # [ci%128, (cc, co)]
        nc.sync.dma_start(dw1s[:], T["dw1t"][:])
        dw2s = wp.tile([RED, 16 * 128], BF16)       # [red, (mch: 2g x 64tap)]
        nc.sync.dma_start(dw2s[:], T["dw2t"][:])
        b1s = wp.tile([HID, 1], F32)
        nc.sync.dma_start(b1s[:], T["b1"][:])
        b2s = wp.tile([HID, 1], F32)
        nc.sync.dma_start(b2s[:], T["b2"][:])
        b3s = wp.tile([128, 4], F32)
        nc.sync.dma_start(b3s[:], T["b3"][:])
        bnscs = wp.tile([RED, 1], F32)
        nc.sync.dma_start(bnscs[:], T["bnsc"][:])
        bnshs = wp.tile([RED, 1], F32)
        nc.sync.dma_start(bnshs[:], T["bnsh"][:])
        idbs = wp.tile([128, 128], BF16)
        nc.sync.dma_start(idbs[:], T["idb"][:])

        # conv input (pre-padded bf16 from host)
        xcp = []
        for cc in range(4):
            t = wp.tile([128, PHW], BF16, name=f"xc{cc}")
            nc.sync.dma_start(t[:], T[f"xc{cc}"][:])
            xcp.append(t)

        # h1/h2 with shifted duplicate in partitions 64-127
        h1d = wp.tile([128, PHW], BF16)
        nc.gpsimd.memset(h1d[:].bitcast(F32), 0.0)
        h2d = wp.tile([128, PHW], BF16)
        nc.gpsimd.memset(h2d[:].bitcast(F32), 0.0)

        with (
            tc.tile_pool(name="gd", bufs=2) as gd,
            tc.tile_pool(name="tfp", bufs=2) as tfp,
            tc.tile_pool(name="fsp", bufs=2) as fsp,
            tc.tile_pool(name="xep", bufs=2) as xep,
            tc.tile_pool(name="xop", bufs=2) as xop,
            tc.tile_pool(name="ftp", bufs=2) as ftp,
            tc.tile_pool(name="ptp", bufs=3) as ptp,
            tc.tile_pool(name="obp", bufs=1) as obp,
            tc.tile_pool(name="cps", bufs=3, space="PSUM") as cps,
            tc.tile_pool(name="aps", bufs=1, space="PSUM") as aps,
        ):
            def conv1_pieces(pb):
                """conv1 for blocks 2pb (psum rows 0-63) and 2pb+1 (rows 64-127,
                col-tiled), split into small pieces for emission interleaving."""
                cell = {}

                def mk(cc, tg):
                    def piece():
                        if "ps" not in cell:
                            cell["ps"] = cps.tile([128, 512], F32, tag="cv", name=f"c1ps{pb}")
                        ps = cell["ps"]
                        for tap in range(3 * tg, 3 * tg + 3):
                            dy, dx = tap // 3, tap % 3
                            nmm = cc * 9 + tap
                            lt = w1s[:, (tap * 4 + cc) * HID:(tap * 4 + cc + 1) * HID]
                            nc.tensor.matmul(
                                ps[0:64, :], lt, crhs(xcp[cc], 16 * pb, dy, dx, 128),
                                start=(nmm == 0), stop=(nmm == 35),
                                tile_position=(0, 0), skip_group_check=True,
                            )
                            nc.tensor.matmul(
                                ps[64:128, :], lt, crhs(xcp[cc], 16 * pb + 8, dy, dx, 128),
                                start=(nmm == 0), stop=(nmm == 35),
                                tile_position=(0, 64), skip_group_check=True,
                            )
                    return piece

                def act_piece():
                    ps = cell["ps"]
                    for half in range(2):
                        r0 = 16 * pb + 8 * half
                        src = ps[64 * half:64 * half + 64, :]
                        nc.scalar.activation(
                            ap_of(h1d, (r0 + 1) * PW + 1, [[PHW, 64], [PW, 8], [1, W]]),
                            src, RELU, bias=b1s[:],
                        )
                        nc.scalar.activation(
                            ap_of(h1d, 64 * PHW + (r0 + 1) * PW, [[PHW, 64], [PW, 8], [1, W]]),
                            src, RELU, bias=b1s[:],
                        )

                return [mk(cc, tg) for cc in range(4) for tg in range(3)] + [act_piece]

            def conv2_pieces(b):
                def piece():
                    r0 = 8 * b
                    ps = cps.tile([128, 512], F32, tag="cv")
                    for dy in range(3):
                        nc.tensor.matmul(
                            ps[0:64, :], w2ps[:, dy * HID:(dy + 1) * HID],
                            ap_of(h1d, (r0 + dy) * PW, [[PHW, 128], [PW, 8], [1, W]]),
                            start=(dy == 0), stop=False,
                        )
                    for dy in range(3):
                        nc.tensor.matmul(
                            ps[0:64, :], w2ss[:, dy * HID:(dy + 1) * HID],
                            ap_of(h1d, (r0 + dy) * PW + 2, [[PHW, 64], [PW, 8], [1, W]]),
                            start=False, stop=(dy == 2),
                        )
                    nc.scalar.activation(
                        ap_of(h2d, (r0 + 1) * PW + 1, [[PHW, 64], [PW, 8], [1, W]]),
                        ps[0:64, :], RELU, bias=b2s[:],
                    )
                    nc.scalar.activation(
                        ap_of(h2d, 64 * PHW + (r0 + 1) * PW, [[PHW, 64], [PW, 8], [1, W]]),
                        ps[0:64, :], RELU, bias=b2s[:],
                    )
                return [piece]

            def conv3_pieces(b, gts):
                def mk(mc):
                    def piece():
                        r0 = 8 * b
                        ps = cps.tile([128, 512], F32, tag="cv")
                        for dy in range(3):
                            nc.tensor.matmul(
                                ps[:], w3ps[:, dy * C + mc * 128:dy * C + (mc + 1) * 128],
                                ap_of(h2d, (r0 + dy) * PW, [[PHW, 128], [PW, 8], [1, W]]),
                                start=(dy == 0), stop=False,
                            )
                        for dy in range(3):
                            nc.tensor.matmul(
                                ps[:], w3ss[:, dy * C + mc * 128:dy * C + (mc + 1) * 128],
                                ap_of(h2d, (r0 + dy) * PW + 2, [[PHW, 64], [PW, 8], [1, W]]),
                                start=False, stop=(dy == 2),
                            )
                        gt = gd.tile([128, 512], BF16, tag=f"g{mc}")
                        nc.scalar.activation(gt[:], ps[:], RELU, bias=b3s[:, mc:mc + 1])
                        gts.append(gt)
                    return piece
                return [mk(mc) for mc in range(4)]

            def dck_pieces(b, gts):
                cell = {}

                def dck1():
                    ps = cps.tile([128, 512], F32, tag="cv")
                    for ccc in range(4):
                        nc.tensor.matmul(
                            ps[:], dw1s[:, ccc * RED:(ccc + 1) * RED], gts[ccc][:],
                            start=(ccc == 0), stop=(ccc == 3),
                        )
                    tft = tfp.tile([RED, 512], BF16, tag="tf")
                    nc.scalar.activation(tft[:], ps[:], RELU, bias=bnshs[:], scale=bnscs[:])
                    cell["tf"] = tft

                def mk(mq):
                    def piece():
                        tft = cell["tf"]
                        fst = fsp.tile([128, 4 * 512], BF16, tag="fs")
                        for q in range(4):
                            m = 4 * mq + q
                            ps2 = cps.tile([128, 512], F32, tag="cv")
                            nc.tensor.matmul(
                                ps2[:], dw2s[:, m * 128:(m + 1) * 128], tft[:],
                                start=True, stop=True,
                            )
                            nc.scalar.activation(fst[:, q * 512:(q + 1) * 512], ps2[:], COPY)
                        nc.sync.dma_start(
                            ap_of(fbuf, (8 * mq) * FB_G + b * 512,
                                  [[FB_G, 2], [PIX, TPAD], [2 * FB_G, 4], [1, 512]]),
                            fst[:],
                        )
                    return piece

                return [dck1] + [mk(mq) for mq in range(4)]

            def load_pieces(b, nxt):
                def xld():
                    # contiguous per-block slabs (host-prepared): 128 fat
                    # descriptors per DMA instead of 4096 thin ones
                    xe = xep.tile([128, XES], BF16, tag="xe")
                    xo = xop.tile([128, XES], BF16, tag="xo")
                    nc.sync.dma_start(
                        xe[:], ap_of(xsb, b * 128 * XES, [[XES, 128], [1, XES]]))
                    nc.sync.dma_start(
                        xo[:], ap_of(xsbo, b * 128 * XES, [[XES, 128], [1, XES]]))
                    nxt["xe"], nxt["xo"] = xe, xo

                def mk_ft(t0c, t1c):
                    def piece():
                        # filter loads go on the scalar-engine HWDGE queue:
                        # ~1.5k thin descriptors per DMA would clog the sync
                        # queue's descriptor generator
                        ntc = t1c - t0c
                        ftt = ftp.tile([128, ntc * 128], BF16, tag=f"ft{t0c}")
                        for rp in range(4):
                            nc.sync.dma_start(
                                ftt[rp * 32:(rp + 1) * 32, :],
                                ap_of(fbuf, t0c * PIX + (b * 8 + 2 * rp) * W,
                                      [[FB_G, G], [PIX, ntc], [1, 128]]),
                            )
                        nxt.setdefault("ft", []).append(ftt)
                    return piece

                return [xld, mk_ft(0, 32), mk_ft(32, NTAP)]

            gts_by_block = {}

            def c3(b):
                return conv3_pieces(b, gts_by_block.setdefault(b, []))

            def dckb(b):
                return dck_pieces(b, gts_by_block[b])

            def stage_pieces(b, nxt):
                """Emission pieces interleaved into apply(b)'s tap loop. The
                filter chain runs two blocks ahead of the apply (D at b+2) so
                the dck2 -> fbuf -> ft round trip has a full block of slack;
                loads for b+1 are emitted before the b+2 fbuf writes so the
                DRAM dependency tracker orders them after the b+1 writes only."""
                pieces = []
                if b == 0:
                    pieces += conv1_pieces(2)
                    pieces += conv2_pieces(2) + conv2_pieces(3)
                    pieces += c3(1) + dckb(1) + c3(2)
                if b == 1:
                    pieces += conv1_pieces(3)
                if b + 4 < NBLK:
                    pieces += conv2_pieces(b + 4)
                if b + 1 < NBLK:
                    pieces += load_pieces(b + 1, nxt)
                if b + 2 < NBLK:
                    pieces += dckb(b + 2)
                if b + 3 < NBLK:
                    pieces += c3(b + 3)
                return pieces

            def apply_block(b, loaded, pieces):
                xe, xo, fts = loaded["xe"], loaded["xo"], loaded["ft"]
                pso = aps.tile([128, 2048], F32, tag="pso")
                # residual init: pso = I @ x_central
                for j in range(4):
                    nc.tensor.matmul(
                        pso[:, j * 512:(j + 1) * 512], idbs[:],
                        ap_of(xe, 3 * XGW + 3 + j * 4 * XGS_GC_E,
                              [[XES, 128], [XGS_GC_E, 4], [XGW, 2], [1, W]]),
                        start=True, stop=False,
                    )
                npc = 0
                for tch, (t0c, t1c) in enumerate(((0, 32), (32, NTAP))):
                    ntc = t1c - t0c
                    ftt = fts[tch]
                    for t in range(t0c, t1c):
                        dy, dx = t // K, t % K
                        if dx % 2 == 0:
                            xsrc, xoff = xe, dy * XGW + dx
                        else:
                            xsrc, xoff = xo, dy * XGW + dx - 1
                        in0 = ap_of(xsrc, xoff,
                                    [[XES, 128], [XGS_GC_E, GC], [XGW, 2], [1, W]])
                        in1 = ap_of(ftt, (t - t0c) * 128,
                                    [[ntc * 128, 128], [0, GC], [W, 2], [1, W]])
                        pt = ptp.tile([128, 2048], BF16, tag="pt")
                        pout = ap_of(pt, 0, [[2048, 128], [128, GC], [W, 2], [1, W]])
                        nc.vector.tensor_tensor(pout, in0, in1, op=MULT)
                        for j in range(4):
                            nc.tensor.matmul(
                                pso[:, j * 512:(j + 1) * 512], idbs[:],
                                pt[:, j * 512:(j + 1) * 512],
                                start=False, stop=(t == NTAP - 1),
                            )
                        # pace next-block stage emission across the tap loop
                        want = (t + 1) * len(pieces) // NTAP
                        while npc < want:
                            pieces[npc]()
                            npc += 1
                while npc < len(pieces):
                    pieces[npc]()
                    npc += 1
                ob = obp.tile([128, 2048], F32, tag="ob")
                nc.scalar.activation(ob[:], pso[:], COPY)
                # output stores on the gpsimd queue so the sync (load) queue
                # never head-of-line blocks on apply completion
                for rp in range(4):
                    nc.gpsimd.dma_start(
                        ap_of(out, (b * 8 + 2 * rp) * W,
                              [[GC * PIX, G], [PIX, GC], [W, 2], [1, W]]),
                        ob[rp * 32:(rp + 1) * 32, :],
                    )

            # ---- pipeline: minimal fill for block 0, then interleave ----
            loaded = {}
            for p in (conv1_pieces(0) + conv2_pieces(0) + conv1_pieces(1)
                      + conv2_pieces(1) + c3(0) + dckb(0) + load_pieces(0, loaded)):
                p()
            for b in range(NBLK):
                nxt = {}
                pieces = stage_pieces(b, nxt)
                apply_block(b, loaded, pieces)
                loaded = nxt


def prep_weights(inputs):
    """Host-side weight transforms shared by all cores."""
    bf = ml_dtypes.bfloat16
    w1 = np.asarray(inputs["w1"], np.float32)   # [64, 512, 3, 3]
    w2 = np.asarray(inputs["w2"], np.float32)
    w3 = np.asarray(inputs["w3"], np.float32)   # [512, 64, 3, 3]
    dck_w1 = np.asarray(inputs["dck_w1"], np.float32)  # [128, 512, 1, 1]
    dck_w2 = np.asarray(inputs["dck_w2"], np.float32)  # [1568, 128, 1, 1]

    def tapify(w):  # [co, ci, 3, 3] -> [9, ci, co]
        return np.ascontiguousarray(w.transpose(2, 3, 1, 0).reshape(9, w.shape[1], w.shape[0]))

    w1sb = tapify(w1).reshape(9, 4, 128, HID).transpose(2, 0, 1, 3).reshape(128, 9 * 4 * HID)

    def pair_split(w9):  # [9, ci(64), co] -> pair [128, 3*co], single [64, 3*co]
        co = w9.shape[2]
        wp = np.zeros((128, 3, co), np.float32)
        ws = np.zeros((64, 3, co), np.float32)
        for dy in range(3):
            wp[0:64, dy] = w9[3 * dy]
            wp[64:128, dy] = w9[3 * dy + 1]
            ws[:, dy] = w9[3 * dy + 2]
        return wp.reshape(128, 3 * co), ws.reshape(64, 3 * co)

    w2pb, w2sb = pair_split(tapify(w2))
    w3pb, w3sb = pair_split(tapify(w3))
    dw1sb = dck_w1.reshape(RED, C).T.reshape(4, 128, RED).transpose(1, 0, 2).reshape(128, 4 * RED)

    bn_g = np.asarray(inputs["bn_gamma"], np.float32)
    bn_b = np.asarray(inputs["bn_beta"], np.float32)
    bn_m = np.asarray(inputs["bn_mean"], np.float32)
    bn_v = np.asarray(inputs["bn_var"], np.float32)
    inv_std = bn_g / np.sqrt(bn_v + 1e-5)
    shift = bn_b - bn_m * inv_std

    dw2 = dck_w2.reshape(G, NTAP, RED)          # [g, t, red]
    dw2p = np.zeros((G, 64, RED), np.float32)
    dw2p[:, :NTAP] = dw2
    # per m-chunk: [red, 2g x 64t]
    dw2t = np.ascontiguousarray(dw2p.reshape(16, 128, RED).transpose(2, 0, 1).reshape(RED, 16 * 128))

    return {
        "w1t": w1sb.astype(bf),
        "b1": np.asarray(inputs["b1"], np.float32).reshape(HID, 1),
        "w2pt": w2pb.astype(bf),
        "w2st": w2sb.astype(bf),
        "b2": np.asarray(inputs["b2"], np.float32).reshape(HID, 1),
        "w3pt": w3pb.astype(bf),
        "w3st": w3sb.astype(bf),
        "b3": np.ascontiguousarray(np.asarray(inputs["b3"], np.float32).reshape(4, 128).T),
        "dw1t": dw1sb.astype(bf),
        "bnsc": inv_std.reshape(RED, 1),
        "bnsh": shift.reshape(RED, 1),
        "dw2t": dw2t.astype(bf),
        "idb": np.eye(128).astype(bf),
    }


def prep_x(xi):
    """Per-core x transforms: padded conv input + padded apply image (bf16)."""
    bf = ml_dtypes.bfloat16
    xi = np.asarray(xi, np.float32)
    xc = np.zeros((4, 128, H + 2, PW), np.float32)
    xc[:, :, 1:H + 1, 1:W + 1] = xi.reshape(4, 128, H, W)
    xgf = np.zeros((G, GC, XGR, XGW), np.float32)
    xgf[:, :, 3:H + 3, 3:W + 3] = xi.reshape(G, GC, H, W)
    xgo = np.zeros_like(xgf)
    xgo[:, :, :, :XGW - 1] = xgf[:, :, :, 1:]
    # per-block contiguous slabs: [b][p=(rp,g)][(gc, 8 rows, 70)]
    xsb = np.empty((NBLK, 4, G, GC, 8, XGW), np.float32)
    xsbo = np.empty_like(xsb)
    for b in range(NBLK):
        for rp in range(4):
            r0 = b * 8 + 2 * rp
            xsb[b, rp] = xgf[:, :, r0:r0 + 8, :]
            xsbo[b, rp] = xgo[:, :, r0:r0 + 8, :]
    m = {f"xc{cc}": np.ascontiguousarray(xc[cc].reshape(128, PHW)).astype(bf) for cc in range(4)}
    m["xsb"] = xsb.reshape(NBLK, 128, XES).astype(bf)
    m["xsbo"] = xsbo.reshape(NBLK, 128, XES).astype(bf)
    return m


def make_in_maps(inputs):
    wmap = prep_weights(inputs)
    x = np.asarray(inputs["x"], np.float32)
    return [{**prep_x(x[i]), **wmap} for i in range(N_CORES)]


_NC_CACHE = {}


def get_nc():
    if "nc" not in _NC_CACHE:
        _NC_CACHE["nc"] = build_nc()
    return _NC_CACHE["nc"]


def kernel(**inputs):
    nc = get_nc()
    in_maps = make_in_maps(inputs)
    res = bass_utils.run_bass_kernel_spmd(nc, in_maps, core_ids=list(range(N_CORES)))
    return np.stack([res.results[i]["out"] for i in range(N_CORES)]).astype(np.float32)


# revision 27
# speedup vs baseline: 1.2893x; 1.0062x over previous
"""Trainium2 Bass kernel for nn_DepthCue (dynamic-filter / CARAFE-style module).

Sharding: data-parallel over batch B=8 across the 8 NeuronCores (one sample
per core).

Per core, row-block pipelined over 8 blocks of 8 image rows:
  - guide network (3x3 convs C->64->64->C) + DCK (1x1 convs + BN/ReLU) run on
    TensorE in bf16 (keeps the PE HAM-warm, 2x the f32r rate). conv2/conv3
    accumulate two taps per matmul via a shifted duplicate of the input in
    partitions 64-127; conv1 packs two row-blocks via column tiling.
  - dynamic-filter apply: partitions = (row-pair, group); per-tap elementwise
    multiply on VectorE in bf16 (2x mode), tap accumulation via
    identity-matmul into PSUM (fp32); the residual x is the PSUM init.
  - block pipeline overlaps the VectorE apply of block b with the guide
    convs of blocks b+1/b+2 on TensorE.
"""

import numpy as np
import ml_dtypes

import concourse.bass as bass
import concourse.bacc as bacc
import concourse.mybir as mybir
from concourse import bass_utils
from concourse.tile import TileContext

F32 = mybir.dt.float32
BF16 = mybir.dt.bfloat16
MULT = mybir.AluOpType.mult
RELU = mybir.ActivationFunctionType.Relu
COPY = mybir.ActivationFunctionType.Copy

N_CORES = 8
C, H, W = 512, 64, 64
HID = 64          # guide-net hidden channels
RED = 128         # DCK reduction channels
G = 32            # groups
GC = 16           # channels per group
K = 7             # dynamic kernel size
NTAP = K * K      # 49
NBLK = 8          # row blocks (8 rows each)
PW = W + 2        # padded width for conv intermediates (66)
PHW = (H + 2) * PW  # 4356
PIX = H * W       # 4096
XGW = W + 6       # apply x padded cols (70)
XGR = H + 6       # apply x padded rows (70)
XGS_GC = XGR * XGW          # 4900
XGS_G = GC * XGS_GC         # 78400
XES = GC * 8 * XGW          # xe per-partition elems (gc, 8 rows, 70) = 8960
XGS_GC_E = 8 * XGW          # xe per-gc stride (560)
TPAD = 64                   # taps padded to 64 in fbuf
FB_G = TPAD * PIX           # fbuf per-group stride (262144)


def ap_of(t, offset, dims):
    """Raw AP over tile/dram tensor t: dims = [[step, count], ...] (dim0 = partition for sbuf)."""
    base = t if isinstance(t, bass.AP) else t[:]
    return bass.AP(tensor=base.tensor, offset=offset, ap=[list(d) for d in dims])


def build_nc():
    nc = bacc.Bacc(trn_type="TRN2", target_bir_lowering=False, debug=False)

    T = {}
    for cc in range(4):
        T[f"xc{cc}"] = nc.dram_tensor(f"xc{cc}", [128, PHW], BF16, kind="ExternalInput").ap()
    T["xsb"] = nc.dram_tensor("xsb", [NBLK, 128, XES], BF16, kind="ExternalInput").ap()
    T["xsbo"] = nc.dram_tensor("xsbo", [NBLK, 128, XES], BF16, kind="ExternalInput").ap()
    T["w1t"] = nc.dram_tensor("w1t", [128, 9 * 4 * HID], BF16, kind="ExternalInput").ap()
    T["b1"] = nc.dram_tensor("b1", [HID, 1], F32, kind="ExternalInput").ap()
    T["w2pt"] = nc.dram_tensor("w2pt", [128, 3 * HID], BF16, kind="ExternalInput").ap()
    T["w2st"] = nc.dram_tensor("w2st", [HID, 3 * HID], BF16, kind="ExternalInput").ap()
    T["b2"] = nc.dram_tensor("b2", [HID, 1], F32, kind="ExternalInput").ap()
    T["w3pt"] = nc.dram_tensor("w3pt", [128, 3 * C], BF16, kind="ExternalInput").ap()
    T["w3st"] = nc.dram_tensor("w3st", [HID, 3 * C], BF16, kind="ExternalInput").ap()
    T["b3"] = nc.dram_tensor("b3", [128, 4], F32, kind="ExternalInput").ap()
    T["dw1t"] = nc.dram_tensor("dw1t", [128, 4 * RED], BF16, kind="ExternalInput").ap()
    T["bnsc"] = nc.dram_tensor("bnsc", [RED, 1], F32, kind="ExternalInput").ap()
    T["bnsh"] = nc.dram_tensor("bnsh", [RED, 1], F32, kind="ExternalInput").ap()
    T["dw2t"] = nc.dram_tensor("dw2t", [RED, 16 * 128], BF16, kind="ExternalInput").ap()
    T["idb"] = nc.dram_tensor("idb", [128, 128], BF16, kind="ExternalInput").ap()
    T["out"] = nc.dram_tensor("out", [C, H, W], F32, kind="ExternalOutput").ap()
    # filters scratch: [g, tap(64), h, w] bf16
    T["fbuf"] = nc.dram_tensor("fbuf", [G, TPAD, H, W], BF16, kind="Internal").ap()

    with TileContext(nc) as tc:
        build_body(nc, tc, T)
    nc.compile()
    return nc


def crhs(src, r0, dy, dx, npart):
    """conv rhs: padded rows r0+dy.., 8 output rows, cols dx.., over npart partitions."""
    return ap_of(src, (r0 + dy) * PW + dx, [[PHW, npart], [PW, 8], [1, W]])


def build_body(nc, tc, T):
    out, fbuf, xsb, xsbo = T["out"], T["fbuf"], T["xsb"], T["xsbo"]

    with tc.tile_pool(name="wp", bufs=1) as wp:
        # ---- persistent weights ----
        w1s = wp.tile([128, 9 * 4 * HID], BF16)     # [ci%128, (tap, cc, co)]
        nc.sync.dma_start(w1s[:], T["w1t"][:])
        w2ps = wp.tile([128, 3 * HID], BF16)        # [dy: taps (3dy, 3dy+1) stacked]
        nc.sync.dma_start(w2ps[:], T["w2pt"][:])
        w2ss = wp.tile([HID, 3 * HID], BF16)        # [dy: tap 3dy+2]
        nc.sync.dma_start(w2ss[:], T["w2st"][:])
        w3ps = wp.tile([128, 3 * C], BF16)
        nc.sync.dma_start(w3ps[:], T["w3pt"][:])
        w3ss = wp.tile([HID, 3 * C], BF16)
        nc.sync.dma_start(w3ss[:], T["w3st"][:])
        dw1s = wp.tile([128, 4 * RED], BF16)        # [ci%128, (cc, co)]
        nc.sync.dma_start(dw1s[:], T["dw1t"][:])
        dw2s = wp.tile([RED, 16 * 128], BF16)       # [red, (mch: 2g x 64tap)]
        nc.sync.dma_start(dw2s[:], T["dw2t"][:])
        b1s = wp.tile([HID, 1], F32)
        nc.sync.dma_start(b1s[:], T["b1"][:])
        b2s = wp.tile([HID, 1], F32)
        nc.sync.dma_start(b2s[:], T["b2"][:])
        b3s = wp.tile([128, 4], F32)
        nc.sync.dma_start(b3s[:], T["b3"][:])
        bnscs = wp.tile([RED, 1], F32)
        nc.sync.dma_start(bnscs[:], T["bnsc"][:])
        bnshs = wp.tile([RED, 1], F32)
        nc.sync.dma_start(bnshs[:], T["bnsh"][:])
        idbs = wp.tile([128, 128], BF16)
        nc.sync.dma_start(idbs[:], T["idb"][:])

        # conv input (pre-padded bf16 from host)
        xcp = []
        for cc in range(4):
            t = wp.tile([128, PHW], BF16, name=f"xc{cc}")
            nc.sync.dma_start(t[:], T[f"xc{cc}"][:])
            xcp.append(t)

        # h1/h2 with shifted duplicate in partitions 64-127
        h1d = wp.tile([128, PHW], BF16)
        nc.gpsimd.memset(h1d[:].bitcast(F32), 0.0)
        h2d = wp.tile([128, PHW], BF16)
        nc.gpsimd.memset(h2d[:].bitcast(F32), 0.0)

        with (
            tc.tile_pool(name="gd", bufs=2) as gd,
            tc.tile_pool(name="tfp", bufs=2) as tfp,
            tc.tile_pool(name="fsp", bufs=2) as fsp,
            tc.tile_pool(name="xep", bufs=2) as xep,
            tc.tile_pool(name="xop", bufs=2) as xop,
            tc.tile_pool(name="ftp", bufs=2) as ftp,
            tc.tile_pool(name="ptp", bufs=3) as ptp,
            tc.tile_pool(name="obp", bufs=1) as obp,
            tc.tile_pool(name="cps", bufs=3, space="PSUM") as cps,
            tc.tile_pool(name="aps", bufs=1, space="PSUM") as aps,
        ):
            def conv1_pieces(pb):
                """conv1 for blocks 2pb (psum rows 0-63) and 2pb+1 (rows 64-127,
                col-tiled), split into small pieces for emission interleaving."""
                cell = {}

                def mk(cc, tg):
                    def piece():
                        if "ps" not in cell:
                            cell["ps"] = cps.tile([128, 512], F32, tag="cv", name=f"c1ps{pb}")
                        ps = cell["ps"]
                        for tap in range(3 * tg, 3 * tg + 3):
                            dy, dx = tap // 3, tap % 3
                            nmm = cc * 9 + tap
                            lt = w1s[:, (tap * 4 + cc) * HID:(tap * 4 + cc + 1) * HID]
                            nc.tensor.matmul(
                                ps[0:64, :], lt, crhs(xcp[cc], 16 * pb, dy, dx, 128),
                                start=(nmm == 0), stop=(nmm == 35),
                                tile_position=(0, 0), skip_group_check=True,
                            )
                            nc.tensor.matmul(
                                ps[64:128, :], lt, crhs(xcp[cc], 16 * pb + 8, dy, dx, 128),
                                start=(nmm == 0), stop=(nmm == 35),
                                tile_position=(0, 64), skip_group_check=True,
                            )
                    return piece

                def act_piece():
                    ps = cell["ps"]
                    for half in range(2):
                        r0 = 16 * pb + 8 * half
                        src = ps[64 * half:64 * half + 64, :]
                        nc.scalar.activation(
                            ap_of(h1d, (r0 + 1) * PW + 1, [[PHW, 64], [PW, 8], [1, W]]),
                            src, RELU, bias=b1s[:],
                        )
                        nc.scalar.activation(
                            ap_of(h1d, 64 * PHW + (r0 + 1) * PW, [[PHW, 64], [PW, 8], [1, W]]),
                            src, RELU, bias=b1s[:],
                        )

                return [mk(cc, tg) for cc in range(4) for tg in range(3)] + [act_piece]

            def conv2_pieces(b):
                def piece():
                    r0 = 8 * b
                    ps = cps.tile([128, 512], F32, tag="cv")
                    for dy in range(3):
                        nc.tensor.matmul(
                            ps[0:64, :], w2ps[:, dy * HID:(dy + 1) * HID],
                            ap_of(h1d, (r0 + dy) * PW, [[PHW, 128], [PW, 8], [1, W]]),
                            start=(dy == 0), stop=False,
                        )
                    for dy in range(3):
                        nc.tensor.matmul(
                            ps[0:64, :], w2ss[:, dy * HID:(dy + 1) * HID],
                            ap_of(h1d, (r0 + dy) * PW + 2, [[PHW, 64], [PW, 8], [1, W]]),
                            start=False, stop=(dy == 2),
                        )
                    nc.scalar.activation(
                        ap_of(h2d, (r0 + 1) * PW + 1, [[PHW, 64], [PW, 8], [1, W]]),
                        ps[0:64, :], RELU, bias=b2s[:],
                    )
                    nc.scalar.activation(
                        ap_of(h2d, 64 * PHW + (r0 + 1) * PW, [[PHW, 64], [PW, 8], [1, W]]),
                        ps[0:64, :], RELU, bias=b2s[:],
                    )
                return [piece]

            def conv3_pieces(b, gts):
                def mk(mc):
                    def piece():
                        r0 = 8 * b
                        ps = cps.tile([128, 512], F32, tag="cv")
                        for dy in range(3):
                            nc.tensor.matmul(
                                ps[:], w3ps[:, dy * C + mc * 128:dy * C + (mc + 1) * 128],
                                ap_of(h2d, (r0 + dy) * PW, [[PHW, 128], [PW, 8], [1, W]]),
                                start=(dy == 0), stop=False,
                            )
                        for dy in range(3):
                            nc.tensor.matmul(
                                ps[:], w3ss[:, dy * C + mc * 128:dy * C + (mc + 1) * 128],
                                ap_of(h2d, (r0 + dy) * PW + 2, [[PHW, 64], [PW, 8], [1, W]]),
                                start=False, stop=(dy == 2),
                            )
                        gt = gd.tile([128, 512], BF16, tag=f"g{mc}")
                        nc.scalar.activation(gt[:], ps[:], RELU, bias=b3s[:, mc:mc + 1])
                        gts.append(gt)
                    return piece
                return [mk(mc) for mc in range(4)]

            def dck_pieces(b, gts):
                cell = {}

                def dck1():
                    ps = cps.tile([128, 512], F32, tag="cv")
                    for ccc in range(4):
                        nc.tensor.matmul(
                            ps[:], dw1s[:, ccc * RED:(ccc + 1) * RED], gts[ccc][:],
                            start=(ccc == 0), stop=(ccc == 3),
                        )
                    tft = tfp.tile([RED, 512], BF16, tag="tf")
                    nc.scalar.activation(tft[:], ps[:], RELU, bias=bnshs[:], scale=bnscs[:])
                    cell["tf"] = tft

                def mk(mq):
                    def piece():
                        tft = cell["tf"]
                        fst = fsp.tile([128, 4 * 512], BF16, tag="fs")
                        for q in range(4):
                            m = 4 * mq + q
                            ps2 = cps.tile([128, 512], F32, tag="cv")
                            nc.tensor.matmul(
                                ps2[:], dw2s[:, m * 128:(m + 1) * 128], tft[:],
                                start=True, stop=True,
                            )
                            nc.scalar.activation(fst[:, q * 512:(q + 1) * 512], ps2[:], COPY)
                        nc.sync.dma_start(
                            ap_of(fbuf, (8 * mq) * FB_G + b * 512,
                                  [[FB_G, 2], [PIX, TPAD], [2 * FB_G, 4], [1, 512]]),
                            fst[:],
                        )
                    return piece

                return [dck1] + [mk(mq) for mq in range(4)]

            def load_pieces(b, nxt):
                def xld():
                    # contiguous per-block slabs (host-prepared): 128 fat
                    # descriptors per DMA instead of 4096 thin ones
                    xe = xep.tile([128, XES], BF16, tag="xe")
                    xo = xop.tile([128, XES], BF16, tag="xo")
                    nc.sync.dma_start(
                        xe[:], ap_of(xsb, b * 128 * XES, [[XES, 128], [1, XES]]))
                    nc.sync.dma_start(
                        xo[:], ap_of(xsbo, b * 128 * XES, [[XES, 128], [1, XES]]))
                    nxt["xe"], nxt["xo"] = xe, xo

                def mk_ft(t0c, t1c):
                    def piece():
                        # filter loads go on the scalar-engine HWDGE queue:
                        # ~1.5k thin descriptors per DMA would clog the sync
                        # queue's descriptor generator
                        ntc = t1c - t0c
                        ftt = ftp.tile([128, ntc * 128], BF16, tag=f"ft{t0c}")
                        for rp in range(4):
                            nc.sync.dma_start(
                                ftt[rp * 32:(rp + 1) * 32, :],
                                ap_of(fbuf, t0c * PIX + (b * 8 + 2 * rp) * W,
                                      [[FB_G, G], [PIX, ntc], [1, 128]]),
                            )
                        nxt.setdefault("ft", []).append(ftt)
                    return piece

                return [xld, mk_ft(0, 32), mk_ft(32, NTAP)]

            gts_by_block = {}

            def c3(b):
                return conv3_pieces(b, gts_by_block.setdefault(b, []))

            def dckb(b):
                return dck_pieces(b, gts_by_block[b])

            def stage_pieces(b, nxt):
                """Emission pieces interleaved into apply(b)'s tap loop. The
                filter chain runs two blocks ahead of the apply (D at b+2) so
                the dck2 -> fbuf -> ft round trip has a full block of slack;
                loads for b+1 are emitted before the b+2 fbuf writes so the
                DRAM dependency tracker orders them after the b+1 writes only."""
                pieces = []
                if b == 0:
                    pieces += conv1_pieces(2)
                    pieces += conv2_pieces(2) + conv2_pieces(3)
                    pieces += c3(1) + dckb(1) + c3(2)
                if b == 1:
                    pieces += conv1_pieces(3)
                if b + 4 < NBLK:
                    pieces += conv2_pieces(b + 4)
                if b + 1 < NBLK:
                    pieces += load_pieces(b + 1, nxt)
                if b + 2 < NBLK:
                    pieces += dckb(b + 2)
                if b + 3 < NBLK:
                    pieces += c3(b + 3)
                return pieces

            def apply_block(b, loaded, pieces=()):
                xe, xo, fts = loaded["xe"], loaded["xo"], loaded["ft"]
                pso = aps.tile([128, 2048], F32, tag="pso")
                # residual init: pso = I @ x_central
                for j in range(4):
                    nc.tensor.matmul(
                        pso[:, j * 512:(j + 1) * 512], idbs[:],
                        ap_of(xe, 3 * XGW + 3 + j * 4 * XGS_GC_E,
                              [[XES, 128], [XGS_GC_E, 4], [XGW, 2], [1, W]]),
                        start=True, stop=False,
                    )
                npc = 0
                for tch, (t0c, t1c) in enumerate(((0, 32), (32, NTAP))):
                    ntc = t1c - t0c
                    ftt = fts[tch]
                    for t in range(t0c, t1c):
                        dy, dx = t // K, t % K
                        if dx % 2 == 0:
                            xsrc, xoff = xe, dy * XGW + dx
                        else:
                            xsrc, xoff = xo, dy * XGW + dx - 1
                        in0 = ap_of(xsrc, xoff,
                                    [[XES, 128], [XGS_GC_E, GC], [XGW, 2], [1, W]])
                        in1 = ap_of(ftt, (t - t0c) * 128,
                                    [[ntc * 128, 128], [0, GC], [W, 2], [1, W]])
                        pt = ptp.tile([128, 2048], BF16, tag="pt")
                        pout = ap_of(pt, 0, [[2048, 128], [128, GC], [W, 2], [1, W]])
                        nc.vector.tensor_tensor(pout, in0, in1, op=MULT)
                        for j in range(4):
                            nc.tensor.matmul(
                                pso[:, j * 512:(j + 1) * 512], idbs[:],
                                pt[:, j * 512:(j + 1) * 512],
                                start=False, stop=(t == NTAP - 1),
                            )
                        # pace next-block stage emission across the tap loop
                        want = (t + 1) * len(pieces) // NTAP
                        while npc < want:
                            pieces[npc]()
                            npc += 1
                while npc < len(pieces):
                    pieces[npc]()
                    npc += 1
                ob = obp.tile([128, 2048], F32, tag="ob")
                nc.scalar.activation(ob[:], pso[:], COPY)
                # output stores on the gpsimd queue so the sync (load) queue
                # never head-of-line blocks on apply completion
                for rp in range(4):
                    nc.gpsimd.dma_start(
                        ap_of(out, (b * 8 + 2 * rp) * W,
                              [[GC * PIX, G], [PIX, GC], [W, 2], [1, W]]),
                        ob[rp * 32:(rp + 1) * 32, :],
                    )

            # ---- pipeline: minimal fill for block 0, then interleave ----
            loaded = {}
            for p in (conv1_pieces(0) + conv2_pieces(0) + conv1_pieces(1)
                      + conv2_pieces(1) + c3(0) + dckb(0) + load_pieces(0, loaded)):
                p()
            # stage work is emitted in dataflow order but numerically
            # deprioritized: the scheduler then treats it as filler for the
            # engines' idle gaps during the apply, instead of clumping it
            # ahead of the apply's accumulate matmuls
            for b in range(NBLK):
                nxt = {}
                with tc.high_priority(offset=-1000000):
                    for p in stage_pieces(b, nxt):
                        p()
                apply_block(b, loaded)
                loaded = nxt


def prep_weights(inputs):
    """Host-side weight transforms shared by all cores."""
    bf = ml_dtypes.bfloat16
    w1 = np.asarray(inputs["w1"], np.float32)   # [64, 512, 3, 3]
    w2 = np.asarray(inputs["w2"], np.float32)
    w3 = np.asarray(inputs["w3"], np.float32)   # [512, 64, 3, 3]
    dck_w1 = np.asarray(inputs["dck_w1"], np.float32)  # [128, 512, 1, 1]
    dck_w2 = np.asarray(inputs["dck_w2"], np.float32)  # [1568, 128, 1, 1]

    def tapify(w):  # [co, ci, 3, 3] -> [9, ci, co]
        return np.ascontiguousarray(w.transpose(2, 3, 1, 0).reshape(9, w.shape[1], w.shape[0]))

    w1sb = tapify(w1).reshape(9, 4, 128, HID).transpose(2, 0, 1, 3).reshape(128, 9 * 4 * HID)

    def pair_split(w9):  # [9, ci(64), co] -> pair [128, 3*co], single [64, 3*co]
        co = w9.shape[2]
        wp = np.zeros((128, 3, co), np.float32)
        ws = np.zeros((64, 3, co), np.float32)
        for dy in range(3):
            wp[0:64, dy] = w9[3 * dy]
            wp[64:128, dy] = w9[3 * dy + 1]
            ws[:, dy] = w9[3 * dy + 2]
        return wp.reshape(128, 3 * co), ws.reshape(64, 3 * co)

    w2pb, w2sb = pair_split(tapify(w2))
    w3pb, w3sb = pair_split(tapify(w3))
    dw1sb = dck_w1.reshape(RED, C).T.reshape(4, 128, RED).transpose(1, 0, 2).reshape(128, 4 * RED)

    bn_g = np.asarray(inputs["bn_gamma"], np.float32)
    bn_b = np.asarray(inputs["bn_beta"], np.float32)
    bn_m = np.asarray(inputs["bn_mean"], np.float32)
    bn_v = np.asarray(inputs["bn_var"], np.float32)
    inv_std = bn_g / np.sqrt(bn_v + 1e-5)
    shift = bn_b - bn_m * inv_std

    dw2 = dck_w2.reshape(G, NTAP, RED)          # [g, t, red]
    dw2p = np.zeros((G, 64, RED), np.float32)
    dw2p[:, :NTAP] = dw2
    # per m-chunk: [red, 2g x 64t]
    dw2t = np.ascontiguousarray(dw2p.reshape(16, 128, RED).transpose(2, 0, 1).reshape(RED, 16 * 128))

    return {
        "w1t": w1sb.astype(bf),
        "b1": np.asarray(inputs["b1"], np.float32).reshape(HID, 1),
        "w2pt": w2pb.astype(bf),
        "w2st": w2sb.astype(bf),
        "b2": np.asarray(inputs["b2"], np.float32).reshape(HID, 1),
        "w3pt": w3pb.astype(bf),
        "w3st": w3sb.astype(bf),
        "b3": np.ascontiguousarray(np.asarray(inputs["b3"], np.float32).reshape(4, 128).T),
        "dw1t": dw1sb.astype(bf),
        "bnsc": inv_std.reshape(RED, 1),
        "bnsh": shift.reshape(RED, 1),
        "dw2t": dw2t.astype(bf),
        "idb": np.eye(128).astype(bf),
    }


def prep_x(xi):
    """Per-core x transforms: padded conv input + padded apply image (bf16)."""
    bf = ml_dtypes.bfloat16
    xi = np.asarray(xi, np.float32)
    xc = np.zeros((4, 128, H + 2, PW), np.float32)
    xc[:, :, 1:H + 1, 1:W + 1] = xi.reshape(4, 128, H, W)
    xgf = np.zeros((G, GC, XGR, XGW), np.float32)
    xgf[:, :, 3:H + 3, 3:W + 3] = xi.reshape(G, GC, H, W)
    xgo = np.zeros_like(xgf)
    xgo[:, :, :, :XGW - 1] = xgf[:, :, :, 1:]
    # per-block contiguous slabs: [b][p=(rp,g)][(gc, 8 rows, 70)]
    xsb = np.empty((NBLK, 4, G, GC, 8, XGW), np.float32)
    xsbo = np.empty_like(xsb)
    for b in range(NBLK):
        for rp in range(4):
            r0 = b * 8 + 2 * rp
            xsb[b, rp] = xgf[:, :, r0:r0 + 8, :]
            xsbo[b, rp] = xgo[:, :, r0:r0 + 8, :]
    m = {f"xc{cc}": np.ascontiguousarray(xc[cc].reshape(128, PHW)).astype(bf) for cc in range(4)}
    m["xsb"] = xsb.reshape(NBLK, 128, XES).astype(bf)
    m["xsbo"] = xsbo.reshape(NBLK, 128, XES).astype(bf)
    return m


def make_in_maps(inputs):
    wmap = prep_weights(inputs)
    x = np.asarray(inputs["x"], np.float32)
    return [{**prep_x(x[i]), **wmap} for i in range(N_CORES)]


_NC_CACHE = {}


def get_nc():
    if "nc" not in _NC_CACHE:
        _NC_CACHE["nc"] = build_nc()
    return _NC_CACHE["nc"]


def kernel(**inputs):
    nc = get_nc()
    in_maps = make_in_maps(inputs)
    res = bass_utils.run_bass_kernel_spmd(nc, in_maps, core_ids=list(range(N_CORES)))
    return np.stack([res.results[i]["out"] for i in range(N_CORES)]).astype(np.float32)
